# revision 67
# baseline (speedup 1.0000x reference)
"""Trainium2 Bass kernel for single-head attention (fp8 DoubleRow edition).

Problem: x[8, 2048, 512], Wq/Wk/Wv[512, 512], bq/bk/bv[512] ->
out[8, 2048, 512] where out = softmax((xWq+bq)(xWk+bk)^T / sqrt(512)) (xWv+bv).

Sharding: data-parallel over batch; each of the 8 NeuronCores does one batch
element.

Numerics (unchanged from the verified baseline): scores = x A x^T with
A = Wq Wk^T; softmax-invariant per-query terms dropped; every big matmul is
fp8e4m3 DoubleRow with value+residual operand pairs (3 passes, ~0.1%
effective precision). rel err ~3e-3 vs the 2e-2 gate; numerical sims show
every cheaper pass configuration exceeds the gate, so the 3-pass structure
is the floor.

This revision is a pure schedule/overhead rework of that baseline:
  - The per-key logit bias c = x.(Wk bq) is folded into t's evacuation bias
    (t8 = fp8(t + w3)) instead of 64 dedicated matmuls + a cT row: the
    scores matmul then produces s + c directly and exp uses a constant bias.
  - bv is folded into v at evacuation (softmax weights sum to 1), removing
    the bv add from the critical output tail.
  - ONE psum pool layout for the whole kernel (ring of 3 [P,512] tiles
    shared by t/v projection halves and score tiles + 4 AV banks + 1 den
    bank = exactly 8 banks), so there is no mid-kernel pool-boundary Drain.
  - Junk f32 matmuls warm the PE p-state ramp while the first DMAs land
    (the cost model halves the clock for ~3us after the PE goes busy, and
    the model grants full speed only after 3us of continuous execution).
  - Input DMAs are chunked and ordered by first use (A8, x8 c0, dx8 c0,
    dA8, Wv8, dWv8, x8/dx8 c1); HWDGE serializes ~625ns per DMA so the
    count is kept moderate.
  - t/v projection halves are interleaved INTO the scores/AV pipeline as
    LAG filler, so phase 1 -> phase 2 has no transition bubble.
  - den matmuls ride AFTER their step's AV (deps long satisfied - no
    PE wait-queue pressure) as DoubleRow K=256 sums, 8/step instead of 16;
    den(m,7) rides at scores time and the group stops at den(m,6), so the
    reciprocal is ready before the last AV and each m-tail evacuates
    immediately after av(m,7).
  - EVERY m-chunk fuses its last TWO AV steps per 128-row quarter
    ([av(m,6), av(m,7)+stop, evacuate, DMA]) and spreads the output DMAs
    over the SP and ACT queues: each m's PSUM banks free a step earlier
    (the next m's AV never waits) and the end tail shortens. m=0's finale
    runs one step later than the rest - at its jp==6 slot the interleaved
    filler has not yet emitted the v8[7] halves (use-before-def garbage
    otherwise; caught as NaN on hardware, invisible to the cost model).

Hardware-verified constraints kept from the baseline: GPSIMD cannot touch
PSUM, DR matmuls need >=128-partition stationaries, fp8/f32r memsets are
rejected (use f32 + tensor_copy), and SBUF->SBUF partition-scatter DMA
gathers return garbage.
"""

import os
import sys

for _p in ("/opt/trn_rl_repo", "/root/.axon_site/_ro/trn_rl_repo"):
    if os.path.isdir(_p) and _p not in sys.path:
        sys.path.append(_p)

import ml_dtypes
import numpy as np

import concourse.bacc as bacc
import concourse.mybir as mybir
import concourse.tile as tile
from concourse.bass_utils import run_bass_kernel_spmd

B = 8
S = 2048
D = 512
P = 128
NT = S // P  # 16 s-tiles
MC = 4  # query chunks of 512
SCALE = 1.0 / float(np.sqrt(D))
SHIFT = 3.0
WS = 32.0  # A/Wv prescale so the weight residuals stay in fp8 normal range
NJUNK = 7  # f32 warmup matmuls riding out the initial DMA wait

F32 = mybir.dt.float32
F16 = mybir.dt.float16
F8 = mybir.dt.float8e4
ACT_ID = mybir.ActivationFunctionType.Identity
ACT_EXP = mybir.ActivationFunctionType.Exp
DR = mybir.MatmulPerfMode.DoubleRow
FP8NP = ml_dtypes.float8_e4m3

_CACHE = {}


def _build_nc():
    nc = bacc.Bacc(None)

    # DRAM layouts mirror the SBUF tiles: [ff, p, u, cols] so each partition
    # line is one contiguous run.
    x8d = nc.dram_tensor("x8", [P, 2, 2, S], F8, kind="ExternalInput")
    dx8d = nc.dram_tensor("dx8", [P, 2, 2, S], F8, kind="ExternalInput")
    A8d = nc.dram_tensor("A8", [P, 2, 2, D], F8, kind="ExternalInput")
    dA8d = nc.dram_tensor("dA8", [P, 2, 2, D], F8, kind="ExternalInput")
    Wv8d = nc.dram_tensor("Wv8", [P, 2, 2, D], F8, kind="ExternalInput")
    dWv8d = nc.dram_tensor("dWv8", [P, 2, 2, D], F8, kind="ExternalInput")
    w3td = nc.dram_tensor("w3t", [P, 4], F32, kind="ExternalInput")
    bvd = nc.dram_tensor("bv2", [1, D], F16, kind="ExternalInput")
    out = nc.dram_tensor("out", [S, D], F16, kind="ExternalOutput")

    with tile.TileContext(nc) as tc:
        lp = nc.allow_low_precision(
            reason="fp8 value+residual pairs carry ~0.1% effective precision"
        )
        lp.__enter__()
        with (
            tc.tile_pool(name="consts", bufs=1) as cp,
            tc.tile_pool(name="tv8", bufs=1) as tv8,
            tc.tile_pool(name="e8p", bufs=7) as e8p,
            tc.tile_pool(name="e16p", bufs=6) as e16p,
            tc.tile_pool(name="tv16", bufs=6) as tv16,
            tc.tile_pool(name="denp", bufs=2) as denp,
            tc.tile_pool(name="outp", bufs=3) as outp,
            tc.tile_pool(name="psS", bufs=3, space="PSUM") as psS,
            tc.tile_pool(name="psO", bufs=1, space="PSUM") as psO,
            tc.tile_pool(name="psDen", bufs=1, space="PSUM") as psDen,
        ):
            # ---- SBUF input tiles: one [P, ff, u, cols] tile per tensor;
            # per-ff views keep the downstream matmul API, while DMAs can
            # slice the parent freely (fewer/finer HWDGE transfers) ----
            parents = {}

            def mk2(pool, name, cols):
                t = pool.tile([P, 2, 2, cols], F8, tag=name, name=name)
                parents[name] = t
                return [t[:, ff] for ff in range(2)]

            x8sb = mk2(cp, "x8", S)
            dx8sb = mk2(cp, "dx8", S)
            A8sb = mk2(cp, "A8", D)
            dA8sb = mk2(cp, "dA8", D)
            Wv8sb = mk2(cp, "Wv8", D)
            dWv8sb = mk2(cp, "dWv8", D)
            w3t_sb = cp.tile([P, 4], F32, tag="w3t", name="w3t_sb")
            bv_sb = cp.tile([P, D], F16, tag="bv", name="bv_sb")

            # warmup: junk f32 matmuls keep the PE busy/ramping while the
            # input DMAs land
            ones32 = cp.tile([P, 2, P], F32, tag="ones32", name="ones32")
            # junk matmuls deliberately read ones32 BEFORE any write: their
            # output is never consumed, so garbage operands are harmless and
            # the PE starts ramping at queue-boot with zero semaphore waits
            junk_ps = psS.tile([P, D], F32, tag="s", name="junk_ps")
            for _ in range(NJUNK):
                nc.tensor.matmul(
                    junk_ps[:, 0:P],
                    ones32[:, 0, :],
                    ones32[:, 0, :],
                    start=True,
                    stop=True,
                )
            nc.gpsimd.memset(ones32[:, 0, :], 1.0)
            nc.vector.memset(ones32[:, 1, :], 1.0)
            shift_sb = cp.tile([P, 1], F32, tag="shift", name="shift_sb")
            nc.vector.memset(shift_sb[:], -SHIFT)
            ones_c8 = cp.tile([P, 2, 1], F8, tag="ones_c8", name="ones_c8")
            nc.vector.tensor_copy(ones_c8[:], ones32[:, :, 0:1])

            # ---- input DMAs, ordered by first use; x8/dx8 split in column
            # halves so early chunks land sooner ----
            HS = S // 2

            def dma(name, dr, ff=None, sl=None):
                # ff=None: both ff halves in one DMA (dram outer-dim
                # transposed into the tile's ff axis); sl: column range
                par = parents[name]
                sl = slice(*sl) if sl is not None else slice(None)
                if ff is None:
                    nc.sync.dma_start(out=par[:, :, :, sl], in_=dr[:, :, :, sl])
                else:
                    nc.sync.dma_start(
                        out=par[:, ff, :, sl], in_=dr[:, ff, :, sl]
                    )

            # HWDGE queue (sync/SP), ordered by first use; chunk sizes pick
            # the balance between transfer-chain and HWDGE-chain latency
            dma("A8", A8d, 0)
            dma("x8", x8d, sl=(0, D))
            dma("dx8", dx8d, sl=(0, D))
            nc.gpsimd.dma_start(out=w3t_sb[:], in_=w3td[:])
            dma("A8", A8d, 1)
            dma("dA8", dA8d)
            dma("x8", x8d, sl=(D, HS))
            dma("dx8", dx8d, sl=(D, HS))
            dma("Wv8", Wv8d)
            dma("dWv8", dWv8d)
            dma("x8", x8d, sl=(HS, S))
            dma("dx8", dx8d, sl=(HS, S))

            # ---- persistent projection outputs ----
            t8sb = [
                tv8.tile([P, 2, S], F8, tag=f"t8_{gg}", name=f"t8_{gg}")
                for gg in range(2)
            ]
            dt8sb = [
                tv8.tile([P, 2, S], F8, tag=f"dt8_{gg}", name=f"dt8_{gg}")
                for gg in range(2)
            ]
            v8sb = [
                tv8.tile([P, 2, D], F8, tag=f"v8_{jp}", name=f"v8_{jp}")
                for jp in range(8)
            ]
            dv8sb = [
                tv8.tile([P, 2, D], F8, tag=f"dv8_{jp}", name=f"dv8_{jp}")
                for jp in range(8)
            ]

            evac_n = [0]

            def pair8(t16h, sl8, dsl8):
                # f16 half -> fp8 value + residual; alternate the copy
                # between Pool (slow but otherwise idle) and DVE
                if evac_n[0] % 2 == 0:
                    nc.gpsimd.tensor_copy(sl8, t16h[:])
                else:
                    nc.vector.tensor_copy(sl8, t16h[:])
                evac_n[0] += 1
                nc.vector.tensor_sub(dsl8, t16h[:], sl8)

            th_ps = {}

            def emit_th(gt, mc, part=None):
                # one [P, 512] half of the t projection: t rows gt*128..,
                # query columns mc*512.. ; part=(pi, ff) emits one matmul so
                # the DMA-gated prologue can follow arrival order; part="ev"
                # evacuates. start/stop flags key on k, not emission order,
                # so the prologue must begin with (0,0) and end with (2,1).
                gg, u = gt // 2, gt % 2
                gsl = slice(gt * P, (gt + 1) * P)
                msl = slice(mc * D, (mc + 1) * D)
                passes = ((A8sb, x8sb), (A8sb, dx8sb), (dA8sb, x8sb))
                if (gt, mc) not in th_ps and part != "ev":
                    th_ps[(gt, mc)] = [psS.tile(
                        [P, D], F32, tag="s", name=f"ps_t{gt}_{mc}"
                    ), 0]
                ps, cnt = th_ps[(gt, mc)]
                if part == "ev":
                    emit = []
                elif part is None:
                    emit = [(pi, ff) for pi in range(3) for ff in range(2)]
                else:
                    emit = [part]
                for pi, ff in emit:
                    stat, mov = passes[pi]
                    nc.tensor.matmul(
                        ps[:],
                        stat[ff][:, :, gsl],
                        mov[ff][:, :, msl],
                        start=(cnt == 0),
                        stop=(cnt == 5),
                        perf_mode=DR,
                    )
                    cnt += 1
                    th_ps[(gt, mc)][1] = cnt
                if part is not None and part != "ev":
                    return
                th_ps.pop((gt, mc))
                ps = ps  # noqa - evac reads the accumulated tile
                t16h = tv16.tile([P, D], F16, tag="t16", name=f"t16_{gt}_{mc}")
                nc.scalar.activation(
                    t16h[:], ps[:], ACT_ID,
                    scale=1.0 / WS, bias=w3t_sb[:, gt : gt + 1],
                )
                pair8(t16h, t8sb[gg][:, u, msl], dt8sb[gg][:, u, msl])

            def emit_vh(st):
                # one [P, 512] half of the v projection: key tile st
                jp, u = st // 2, st % 2
                ssl = slice(st * P, (st + 1) * P)
                ps = psS.tile([P, D], F32, tag="s", name=f"ps_v{st}")
                k = 0
                for stat, mov in ((x8sb, Wv8sb), (dx8sb, Wv8sb), (x8sb, dWv8sb)):
                    for ff in range(2):
                        nc.tensor.matmul(
                            ps[:],
                            stat[ff][:, :, ssl],
                            mov[ff][:],
                            start=(k == 0),
                            stop=(k == 5),
                            perf_mode=DR,
                        )
                        k += 1
                v16h = tv16.tile([P, D], F16, tag="t16", name=f"v16_{st}")
                nc.scalar.activation(v16h[:], ps[:], ACT_ID, scale=1.0 / WS)
                # bv folds into v (softmax weights sum to 1)
                nc.vector.tensor_add(v16h[:], v16h[:], bv_sb[:])
                pair8(v16h, v8sb[jp][:, u, :], dv8sb[jp][:, u, :])

            # ---- scores -> exp pair -> AV + den, software-pipelined ----
            state = {}

            def open_m(m):
                state[m] = {
                    "ps_o": [
                        psO.tile([P, D], F32, tag=f"o{t}", name=f"ps_o{t}_{m}")
                        for t in range(4)
                    ],
                    "ps_den": psDen.tile([P, 4], F32, tag="dn", name=f"ps_den{m}"),
                    "pend": [],
                }

            def emit_s(m, jp):
                st = state[m]
                e8t = e8p.tile([P, 2, D], F8, tag="e8", name=f"e8_{m}_{jp}")
                de8t = e8p.tile([P, 2, D], F8, tag="de8", name=f"de8_{m}_{jp}")
                e16 = e16p.tile([P, 2, D], F16, tag="e16", name=f"e16_{m}_{jp}")
                msl = slice(m * D, (m + 1) * D)
                for u in range(2):
                    j16 = jp * 2 + u
                    jsl = slice(j16 * P, (j16 + 1) * P)
                    ps_s = psS.tile([P, D], F32, tag="s", name=f"ps_s{m}_{j16}")
                    k = 0
                    for stat, mov in (
                        (x8sb, t8sb),
                        (x8sb, dt8sb),
                        (dx8sb, t8sb),
                    ):
                        for ff in range(2):
                            nc.tensor.matmul(
                                ps_s[:],
                                stat[ff][:, :, jsl],
                                mov[ff][:, :, msl],
                                start=(k == 0),
                                stop=(k == 5),
                                perf_mode=DR,
                            )
                            k += 1
                    nc.scalar.activation(
                        e16[:, u, :], ps_s[:], ACT_EXP,
                        scale=SCALE, bias=shift_sb[:],
                    )
                nc.vector.tensor_copy(e8t[:], e16[:])
                nc.vector.tensor_sub(de8t[:], e16[:], e8t[:])
                st["pend"].append((jp, e8t, de8t))
                if jp == 7:
                    # the last key-tile's den rides right here (its e8 is
                    # hot); the group then stops at den(m,6), pulling the
                    # reciprocal off the output-tail critical path
                    emit_den(m, 7, e8t, de8t)

            def emit_av_mm(m, jp, e8t, de8t, tqs=range(4)):
                st = state[m]
                for tq in tqs:
                    sl = slice(tq * P, (tq + 1) * P)
                    for pi, (stat, mov) in enumerate((
                        (e8t, v8sb[jp]),
                        (de8t, v8sb[jp]),
                        (e8t, dv8sb[jp]),
                    )):
                        nc.tensor.matmul(
                            st["ps_o"][tq][:],
                            stat[:, :, sl],
                            mov[:],
                            start=(jp == 0 and pi == 0),
                            stop=(jp == 7 and pi == 2),
                            perf_mode=DR,
                        )

            def emit_den(m, jp, e8t, de8t):
                # den rides AFTER av: its deps are long satisfied, so it
                # never clogs the PE wait queue. DR sums 256 keys/instr.
                st = state[m]
                for tq in range(4):
                    sl = slice(tq * P, (tq + 1) * P)
                    for si, src in enumerate((e8t, de8t)):
                        nc.tensor.matmul(
                            st["ps_den"][:, tq : tq + 1],
                            src[:, :, sl],
                            ones_c8[:],
                            start=(jp == 0 and tq == 0 and si == 0),
                            stop=(jp == 6 and si == 1),
                            perf_mode=DR,
                            skip_group_check=True,
                        )
                if jp == 6:
                    rec_sb = denp.tile([P, 4], F32, tag="rs", name=f"rs{m}")
                    nc.vector.reciprocal(rec_sb[:], st["ps_den"][:])
                    st["rec_sb"] = rec_sb

            def evac_tq(st, o_sb, tq):
                osl = o_sb[:, tq, :]
                rc = st["rec_sb"][:, tq : tq + 1]
                # spread across ACT and DVE so the banks release in parallel
                if tq in (1, 2):
                    nc.scalar.activation(osl, st["ps_o"][tq][:], ACT_ID, scale=rc)
                else:
                    nc.vector.tensor_scalar_mul(osl, st["ps_o"][tq][:], rc)

            # ---- the unified emission stream ----
            seq = [(m_, jp_) for m_ in range(MC) for jp_ in range(8)]
            LAG = 4
            filler = (
                [[("VH", 0), ("VH", 1)], [("VH", 2), ("VH", 3)],
                 [("TH", 0, 2), ("TH", 1, 2)], [("TH", 2, 2), ("TH", 3, 2)],
                 [("VH", 4), ("VH", 5)], [("VH", 6), ("VH", 7)],
                 [("TH", 0, 3), ("TH", 1, 3)], [("TH", 2, 3), ("TH", 3, 3)],
                 [("VH", 8), ("VH", 9)], [("VH", 10), ("VH", 11)],
                 [("VH", 12), ("VH", 13)], [("VH", 14), ("VH", 15)]]
            )

            # DMA-gated prologue: (pass, ff)-interleave the first 3 t-halves
            # (the psS ring depth) following the DMA arrival order
            for part in ((0, 0), (1, 0), (0, 1), (1, 1), (2, 0), (2, 1)):
                for gt in range(3):
                    emit_th(gt, 0, part=part)
            emit_th(0, 0, part="ev")
            # bv rides SWDGE behind the first Pool evac copy (just emitted),
            # keeping its transfer out of the critical input window
            nc.gpsimd.dma_start(
                out=bv_sb[:], in_=bvd[0, :].partition_broadcast(P)
            )
            emit_th(1, 0, part="ev")
            emit_th(2, 0, part="ev")
            emit_th(3, 0)
            for part in ((0, 0), (0, 1), (2, 0), (2, 1), (1, 0), (1, 1)):
                for gt in range(3):
                    emit_th(gt, 1, part=part)
            for gt in range(3):
                emit_th(gt, 1, part="ev")
            emit_th(3, 1)

            open_m(0)
            for k in range(len(seq) + LAG):
                if k < len(filler):
                    for it in filler[k]:
                        if it[0] == "TH":
                            emit_th(it[1], it[2])
                        else:
                            emit_vh(it[1])
                if k < len(seq):
                    sm, sj = seq[k]
                    if sj == 0 and sm > 0:
                        open_m(sm)
                    emit_s(sm, sj)
                if k >= LAG:
                    m, jp = seq[k - LAG]
                    # m=0's finale runs one step later (jp==7 slot): at its
                    # jp==6 slot the filler has not yet written v8[7]
                    late0 = m == 0
                    if (jp == 7 and not late0) or (jp == 6 and late0):
                        continue
                    st = state[m]
                    if jp == 6 or jp == 7:
                        # fused m-finale for EVERY m: den(m,6)+stop -> recip,
                        # then per 128-row quarter [av(m,6), av(m,7)+stop,
                        # evacuate, DMA]. The m's PSUM banks free a full step
                        # earlier, so the next m's AV never waits on them.
                        jp6, e8t6, de8t6 = st["pend"].pop(0)
                        jp7, e8t7, de8t7 = st["pend"].pop(0)
                        emit_den(m, jp6, e8t6, de8t6)
                        o_sb = outp.tile(
                            [P, 4, D], F16, tag="osb", name=f"o{m}"
                        )
                        for tq in range(4):
                            emit_av_mm(m, jp6, e8t6, de8t6, tqs=(tq,))
                            emit_av_mm(m, jp7, e8t7, de8t7, tqs=(tq,))
                            evac_tq(st, o_sb, tq)
                            it = m * 4 + tq
                            q = (nc.sync, nc.sync, nc.scalar, nc.sync)[tq]
                            q.dma_start(
                                out=out[it * P : (it + 1) * P, :],
                                in_=o_sb[:, tq, :],
                            )
                        state.pop(m)
                    else:
                        jpp, e8t, de8t = st["pend"].pop(0)
                        emit_av_mm(m, jpp, e8t, de8t)
                        emit_den(m, jpp, e8t, de8t)

        lp.__exit__(None, None, None)

    nc.finalize()
    return nc


def _q8(a):
    return np.ascontiguousarray(a).astype(FP8NP)


def _tile4(a, cols):
    """[512, cols] -> [128, 2, 2, cols] matching the SBUF tile layout."""
    return np.ascontiguousarray(
        a.reshape(2, 2, P, cols).transpose(2, 0, 1, 3)
    )


def _prep_weights(Wq, bq, Wk, bk, Wv, bv):
    Wq = np.asarray(Wq, dtype=np.float64)
    Wk = np.asarray(Wk, dtype=np.float64)
    A = (Wq @ Wk.T).astype(np.float32)
    w3 = (Wk @ np.asarray(bq, dtype=np.float64)).astype(np.float32)
    A8 = _q8(WS * A)
    dA8 = _q8(WS * A - A8.astype(np.float32))
    Wv32 = np.asarray(Wv, dtype=np.float32)
    Wv8 = _q8(WS * Wv32)
    dWv8 = _q8(WS * Wv32 - Wv8.astype(np.float32))
    bv16 = np.asarray(bv, dtype=np.float16)
    return {
        "A8": _tile4(A8, D),
        "dA8": _tile4(dA8, D),
        "Wv8": _tile4(Wv8, D),
        "dWv8": _tile4(dWv8, D),
        "w3t": np.ascontiguousarray(w3.reshape(4, P).T),
        "bv2": np.ascontiguousarray(bv16.reshape(1, D)),
    }


def kernel(x, Wq, bq, Wk, bk, Wv, bv):
    x = np.asarray(x, dtype=np.float32)
    wargs = _prep_weights(Wq, bq, Wk, bk, Wv, bv)

    if "nc" not in _CACHE:
        _CACHE["nc"] = _build_nc()
    nc = _CACHE["nc"]

    in_maps = []
    for b in range(B):
        xT = np.ascontiguousarray(x[b].T)
        x8 = xT.astype(FP8NP)
        dx8 = (xT - x8.astype(np.float32)).astype(FP8NP)
        in_maps.append(
            {"x8": _tile4(x8, S), "dx8": _tile4(dx8, S), **wargs}
        )

    try:
        res = run_bass_kernel_spmd(nc, in_maps, list(range(B)))
    except Exception:
        # transient device wedge (e.g. NRT_EXEC_UNIT_UNRECOVERABLE) - retry
        import time as _time

        _time.sleep(5)
        res = run_bass_kernel_spmd(nc, in_maps, list(range(B)))
    return np.stack(
        [np.asarray(res.results[b]["out"]) for b in range(B)]
    ).astype(np.float32)


if __name__ == "__main__":
    rng = np.random.default_rng(0)
    inputs = {
        "x": rng.standard_normal((B, S, D), dtype=np.float32),
        "Wq": rng.standard_normal((D, D), dtype=np.float32) / np.sqrt(D),
        "bq": rng.standard_normal(D).astype(np.float32) * 0.01,
        "Wk": rng.standard_normal((D, D), dtype=np.float32) / np.sqrt(D),
        "bk": rng.standard_normal(D).astype(np.float32) * 0.01,
        "Wv": rng.standard_normal((D, D), dtype=np.float32) / np.sqrt(D),
        "bv": rng.standard_normal(D).astype(np.float32) * 0.01,
    }
    got = kernel(**inputs)
    print("kernel output", got.shape, got.dtype)
